# revision 1
# baseline (speedup 1.0000x reference)
"""GAT message-passing kernel for 8 trn2 NeuronCores.

Math (reference):
    Wx = x @ W;  s1 = Wx@a1/sqrt(2D);  s2 = Wx@a2/sqrt(2D)   (per t)
    weight = softmax_m(lrelu(s1[m] + s2[n]));  agg = lrelu(weight @ Wx)
    out = x - agg

Key identities:
  * lrelu(v) = max(v, 0.01v) and exp monotone =>
        exp(lrelu(s1+s2)) = max(exp(s1+s2), exp(0.01(s1+s2)))
  * softmax is invariant to per-n rescaling; dividing by exp(s2[n]):
        E~[m,n] = max(E1[m], F1[m] * r[n])
    with E1=exp(s1), F1=exp(0.01 s1), r=exp(-0.99 s2) - O(N) exps only.
    The O(N^2) score tile is ONE fused DVE tensor_scalar:
        (r_b mult F1col) max E1col.
  * softmax denominator folded into the aggregation matmul as a ones
    column appended to Wx.
  * out = x - lrelu(agg) = min(x - agg, x - 0.01*agg).

Sharding: 8 cores = 4 t-slices x 2 N-halves; each core aggregates over all
4096 source nodes for its own (t, 2048 dest nodes).
"""

import sys

if "/opt/trn_rl_repo" not in sys.path:
    sys.path.insert(0, "/opt/trn_rl_repo")

import numpy as np

N, T, D = 4096, 4, 128
P = 128
HALF = N // 2            # 2048 dest nodes per core
MT = N // P              # 32 m tiles
NT = HALF // P           # 16 own n tiles
NQ = HALF // 512         # 4 n chunks of 512
SCALE_INV = 1.0 / 16.0   # 1/sqrt(2*128)

# packed input column offsets: [params | xT | xn]
XCH = N // 4             # 1024
C_PRM = 0
C_XT = 2 * D + 2         # 258
C_XN = C_XT + N          # 4354
C_END = C_XN + HALF      # 6402

_CACHE = {}


def _build():
    import concourse.mybir as mybir
    from concourse import bacc
    from concourse.tile import TileContext

    f32 = mybir.dt.float32
    bf16 = mybir.dt.bfloat16
    Alu = mybir.AluOpType
    Act = mybir.ActivationFunctionType

    nc = bacc.Bacc()
    xin = nc.declare_dram_parameter("xin", [P, C_END], f32, isOutput=False)
    out = nc.declare_dram_parameter("out", [HALF, D], f32, isOutput=True)

    with TileContext(nc) as tc:
        with (
            tc.tile_pool(name="const", bufs=1) as cpool,
            tc.tile_pool(name="epool", bufs=12) as epool,
            tc.tile_pool(name="fpool", bufs=4) as fpool,
            tc.tile_pool(name="opool", bufs=4) as opool,
        ):
            # ---- input DMAs: 4 xT chunks (so projection starts early),
            # then the rest (xn + params) ----
            # chunk 0 carries the params + first quarter of xT
            px0 = cpool.tile([P, C_XT + XCH], f32)
            nc.sync.dma_start(px0[:, :], xin[:, 0 : C_XT + XCH])
            prm = px0[:, 0:C_XT]
            xts = [px0[:, C_XT : C_XT + XCH]]
            for ch in range(1, 4):
                xt_c = cpool.tile([P, XCH], f32, name=f"xt{ch}", tag=f"xt{ch}")
                nc.sync.dma_start(
                    xt_c[:, :], xin[:, C_XT + ch * XCH : C_XT + (ch + 1) * XCH]
                )
                xts.append(xt_c)
            xn_sb = cpool.tile([P, HALF], f32)
            nc.gpsimd.dma_start(xn_sb[:, :], xin[:, C_XN:C_END])
            Wm_sb = prm[:, 0:D]
            WT_sb = prm[:, D : 2 * D]
            av_sb = prm[:, 2 * D : 2 * D + 2]

            wx = cpool.tile([P, MT * (D + 1)], bf16)
            E1 = cpool.tile([P, MT], f32)
            F1 = cpool.tile([P, MT], f32)
            r_b = cpool.tile([P, HALF], bf16)

            with tc.tile_pool(name="ppsum", bufs=2, space="PSUM") as ppool:
                # ---- wproj = [W | w1 | w2] ----
                wproj = cpool.tile([P, D + 2], f32)
                nc.scalar.activation(wproj[:, :D], Wm_sb, Act.Copy)
                w_ps = ppool.tile([P, 2], f32, tag="ps", name="w_ps")
                nc.tensor.matmul(w_ps[:, :], WT_sb, av_sb, start=True, stop=True)
                nc.scalar.activation(
                    wproj[:, D : D + 2], w_ps[:, :], Act.Copy, scale=SCALE_INV
                )
                ones_col = cpool.tile([P, 1], f32)
                nc.scalar.activation(
                    ones_col[:, :], prm[:, 0:1], Act.Copy, scale=0.0, bias=1.0
                )

                # ---- r_b[p, n] = exp(-0.99 * s2[n]) for all p, via one
                # rank-1 stationary (w2 broadcast along free dim): a single
                # full-shape matmul per 512-chunk computes s2 replicated
                # across all 128 partitions; exp lands straight in r_b ----
                w2b = cpool.tile([P, P], f32)
                nc.vector.tensor_scalar(
                    w2b[:, :], Wm_sb, 0.0, wproj[:, D + 1 : D + 2],
                    Alu.mult, Alu.add,
                )
                for q in range(NQ):
                    rb_ps = ppool.tile([P, 512], f32, tag="ps", name="rb_ps")
                    nc.tensor.matmul(
                        rb_ps[:, :],
                        w2b[:, :],
                        xts[q // 2][:, (q % 2) * 512 : (q % 2) * 512 + 512],
                        start=True,
                        stop=True,
                    )
                    nc.scalar.activation(
                        r_b[:, q * 512 : (q + 1) * 512],
                        rb_ps[:, :],
                        Act.Exp,
                        scale=-0.99,
                    )

                # ---- projection: wx = [Wx(bf16) | 1] per mt, E1/F1 ----
                for mt in range(MT):
                    p_ps = ppool.tile(
                        [P, D + 2], f32, tag="pp", name="p_ps", bufs=6
                    )
                    nc.tensor.matmul(
                        p_ps[:, :],
                        xts[mt // 8][:, (mt % 8) * P : (mt % 8) * P + P],
                        wproj[:, :],
                        start=True,
                        stop=True,
                    )
                    base = mt * (D + 1)
                    nc.vector.tensor_copy(wx[:, base : base + D], p_ps[:, :D])
                    nc.scalar.activation(
                        wx[:, base + D : base + D + 1], ones_col[:, :], Act.Copy
                    )
                    nc.scalar.activation(
                        E1[:, mt : mt + 1], p_ps[:, D : D + 1], Act.Exp
                    )
                    nc.scalar.activation(
                        F1[:, mt : mt + 1], p_ps[:, D : D + 1], Act.Exp, scale=0.01
                    )

            # ---- main: score tiles + aggregation (double-buffered acc) ----
            with tc.tile_pool(name="mpsum", bufs=2, space="PSUM") as mpool:
                def finalize(q, acc, o_q):
                    for j in range(4):
                        nt = q * 4 + j
                        rz = fpool.tile([P, 1], f32, tag="rz", name="rz")
                        nc.vector.reciprocal(rz[:, :], acc[j][:, D : D + 1])
                        # lrelu(agg) in one ACT op: Lrelu(rz * numer), slope 0.01
                        lr = fpool.tile([P, D], f32, tag="lr", name="lr")
                        nc.scalar.activation(
                            lr[:, :],
                            acc[j][:, :D],
                            Act.Lrelu,
                            scale=rz[:, :],
                            alpha=0.01,
                        )
                        nc.vector.tensor_tensor(
                            o_q[:, j * D : (j + 1) * D],
                            xn_sb[:, nt * D : (nt + 1) * D],
                            lr[:, :],
                            Alu.subtract,
                        )
                    out_view = out[q * 512 : (q + 1) * 512, :].rearrange(
                        "(j p) d -> p j d", p=P
                    )
                    nc.sync.dma_start(
                        out_view, o_q.rearrange("p (j d) -> p j d", j=4)
                    )

                pending = None
                for q in range(NQ):
                    acc = [
                        mpool.tile([P, D + 1], f32, tag=f"acc{j}", name=f"acc{j}")
                        for j in range(4)
                    ]
                    o_q = opool.tile([P, 4 * D], f32, name="o_q")
                    for mt in range(MT):
                        et = epool.tile([P, 512], bf16, name="et")
                        nc.vector.tensor_scalar(
                            et[:, :],
                            r_b[:, q * 512 : (q + 1) * 512],
                            F1[:, mt : mt + 1],
                            E1[:, mt : mt + 1],
                            Alu.mult,
                            Alu.max,
                        )
                        if mt == 6 and pending is not None:
                            finalize(*pending)
                            pending = None
                        base = mt * (D + 1)
                        for j in range(4):
                            nc.tensor.matmul(
                                acc[j][:, :],
                                et[:, j * P : (j + 1) * P],
                                wx[:, base : base + D + 1],
                                start=(mt == 0),
                                stop=(mt == MT - 1),
                            )
                    pending = (q, acc, o_q)
                finalize(*pending)

    nc.compile()
    return nc


def _prep_inputs(x, W, a1, a2):
    """Per-core packed input. Core c: t = c//2, n-half h = c%2.

    xT is host-rotated so the core's own 2048 dest columns come first
    (a rotation does not change a sum over all source nodes).
    """
    x = np.asarray(x, dtype=np.float32)
    W = np.ascontiguousarray(np.asarray(W, dtype=np.float32))
    WT = np.ascontiguousarray(W.T)
    av = np.ascontiguousarray(
        np.stack([np.asarray(a1, np.float32), np.asarray(a2, np.float32)], axis=1)
    )
    in_maps = []
    for c in range(8):
        t, h = c // 2, c % 2
        xt = x[:, t, :].T  # [D, N]
        if h == 1:
            xt = np.concatenate([xt[:, HALF:], xt[:, :HALF]], axis=1)
        xn = x[h * HALF : (h + 1) * HALF, t, :]  # [2048, 128]
        xn_packed = xn.reshape(NT, P, D).transpose(1, 0, 2).reshape(P, NT * D)
        xin = np.concatenate([W, WT, av, xt, xn_packed], axis=1)
        in_maps.append({"xin": np.ascontiguousarray(xin)})
    return in_maps


def _run(x, W, a1, a2, trace=False):
    from concourse.bass_utils import run_bass_kernel_spmd

    key = "nc"
    if key not in _CACHE:
        _CACHE[key] = _build()
    nc = _CACHE[key]
    in_maps = _prep_inputs(x, W, a1, a2)
    res = run_bass_kernel_spmd(nc, in_maps, list(range(8)), trace=trace)
    out_full = np.empty((N, T, D), dtype=np.float32)
    for c in range(8):
        t, h = c // 2, c % 2
        out_full[h * HALF : (h + 1) * HALF, t, :] = res.results[c]["out"]
    return out_full, res


def kernel(x, W, a1, a2):
    out, _ = _run(x, W, a1, a2, trace=False)
    return out

